# revision 14
# baseline (speedup 1.0000x reference)
"""Trainium2 Bass kernel for nn_Match_Decoder (single-step GRU decoder with
Luong attention and a 32000-way output projection), tensor-parallel over 8
NeuronCores.

Sharding (hardcoded):
  - GRU gate rows: each core owns 128 rows of each of the r/z/n gate blocks
    of W_ih/W_hh (384 rows of 3072). Partials are AllGathered (6KB).
  - Attention: encoder positions sharded 512/core. Local softmax numerator +
    (max, sum) AllGather (64B) for the global rescale; context partials
    AllReduced (4KB).
  - Output projection: vocab rows sharded 4000/core; log_softmax via a
    second (max, sum) AllGather (64B).
  - Embedding table replicated; the one needed row is fetched on-device by
    indirect DMA.
"""

import numpy as np

import concourse.bass as bass
import concourse.mybir as mybir
import concourse.tile as tile
from concourse import bacc
from concourse.bass_utils import run_bass_kernel_spmd
from concourse.masks import make_identity

F32 = mybir.dt.float32
AF = mybir.ActivationFunctionType

NCORES = 8
H = 1024
I2 = 2 * H          # 2048
VOCAB = 32000
VS = VOCAB // NCORES      # 4000 vocab rows per core
SRC = 4096
SS = SRC // NCORES        # 512 source positions per core
GR = 3 * 128              # 384 gru rows per core (128 per gate)
NEG = -1.0e30

# number of full 128-row m-tiles in the 4000-row vocab shard (31 full + 32)
VM_FULL = VS // 128       # 31
VM_REM = VS - VM_FULL * 128   # 32


def build_program():
    nc = bacc.Bacc("TRN2", target_bir_lowering=False, debug=False,
                   num_devices=NCORES)

    # ---- I/O params (per-core shards prepared on host) ----
    word = nc.declare_dram_parameter("word", [1, 1], mybir.dt.int32, isOutput=False)
    lc = nc.declare_dram_parameter("lc", [H], F32, isOutput=False)
    hprev = nc.declare_dram_parameter("hprev", [H], F32, isOutput=False)
    emb = nc.declare_dram_parameter("emb", [VOCAB, H], F32, isOutput=False)
    wihT = nc.declare_dram_parameter("wihT", [I2, GR], F32, isOutput=False)
    whhT = nc.declare_dram_parameter("whhT", [H, GR], F32, isOutput=False)
    gbias = nc.declare_dram_parameter("gbias", [2 * GR], F32, isOutput=False)
    encT = nc.declare_dram_parameter("encT", [H, SS], F32, isOutput=False)
    encn = nc.declare_dram_parameter("encn", [SS, H], F32, isOutput=False)
    woutT = nc.declare_dram_parameter("woutT", [I2, VS], F32, isOutput=False)
    bout = nc.declare_dram_parameter("bout", [VS], F32, isOutput=False)

    out_logits = nc.declare_dram_parameter("out_logits", [VS], F32, isOutput=True)
    out_h = nc.declare_dram_parameter("out_h", [H], F32, isOutput=True)
    out_ctx = nc.declare_dram_parameter("out_ctx", [H], F32, isOutput=True)
    out_attn = nc.declare_dram_parameter("out_attn", [SS], F32, isOutput=True)

    # ---- internal DRAM bounce buffers for collectives ----
    g_in = nc.dram_tensor("g_in", [6 * 128], F32)
    g_out = nc.dram_tensor("g_out", [NCORES * 6 * 128], F32, addr_space="Shared")
    st_in = nc.dram_tensor("st_in", [2], F32)
    st_out = nc.dram_tensor("st_out", [2 * NCORES], F32, addr_space="Shared")
    ctx_in = nc.dram_tensor("ctx_in", [H], F32)
    ctx_out = nc.dram_tensor("ctx_out", [H], F32, addr_space="Shared")
    ls_in = nc.dram_tensor("ls_in", [2], F32)
    ls_out = nc.dram_tensor("ls_out", [2 * NCORES], F32, addr_space="Shared")

    GROUPS = [list(range(NCORES))]

    with tile.TileContext(nc, num_cores=NCORES) as tc:
        with (
            tc.tile_pool(name="const", bufs=1) as pconst,
            tc.tile_pool(name="small", bufs=1) as ps,
            tc.tile_pool(name="big", bufs=1) as pb,
            tc.tile_pool(name="wout", bufs=5) as pw,
            tc.tile_pool(name="psum", bufs=1, space="PSUM") as pp,
            tc.tile_pool(name="psum2", bufs=2, space="PSUM") as pp2,
        ):
            ident = pconst.tile([128, 128], F32, tag="ident")
            make_identity(nc, ident[:])
            ones_row = pconst.tile([1, 128], F32, tag="ones_r")
            nc.vector.memset(ones_row[:], 1.0)
            ones_col = pconst.tile([128, 1], F32, tag="ones_c")
            nc.vector.memset(ones_col[:], 1.0)

            def bcast128(val_ap, name):
                """broadcast a [1,1] value to a [128,1] sbuf tile (via PE)."""
                t = pp.tile([128, 1], F32, tag="tp")
                nc.tensor.matmul(t[:], lhsT=ones_row[:], rhs=val_ap)
                sb = ps.tile([128, 1], F32, tag=name)
                nc.vector.tensor_copy(sb[:], t[:])
                return sb

            def psum_sum(col128, name):
                """sum over partitions of a [128,1] -> [1,1] (psum AP)."""
                t = pp.tile([1, 1], F32, tag="tp")
                nc.tensor.matmul(t[:], lhsT=ones_col[:], rhs=col128)
                return t

            def part_max(src_ap, name):
                """max over a [128, n] AP -> [1,1] sbuf tile."""
                m128 = ps.tile([128, 1], F32, tag=name + "_m128")
                nc.vector.reduce_max(m128[:], src_ap, axis=mybir.AxisListType.X)
                tp = pp.tile([1, 128], F32, tag="tp")
                nc.tensor.transpose(tp[:], m128[:], ident[:])
                row = ps.tile([1, 128], F32, tag=name + "_row")
                nc.vector.tensor_copy(row[:], tp[:])
                m = ps.tile([1, 1], F32, tag=name + "_m")
                nc.vector.reduce_max(m[:], row[:], axis=mybir.AxisListType.X)
                return m

            # ================= embedding lookup =================
            wsb = ps.tile([1, 1], mybir.dt.int32, tag="wsb")
            nc.scalar.dma_start(wsb[:], word[:, :])
            idx2 = ps.tile([2, 1], mybir.dt.int32, tag="idx2")
            nc.gpsimd.partition_broadcast(idx2[:], wsb[:], channels=2)
            xrow = ps.tile([2, H], F32, tag="xrow")
            nc.gpsimd.indirect_dma_start(
                out=xrow[:],
                out_offset=None,
                in_=emb[:],
                in_offset=bass.IndirectOffsetOnAxis(ap=idx2[:, 0:1], axis=0),
            )

            # natural-layout [24,128] = [x(8); lc(8); h(8)] rows of 128
            nat24 = ps.tile([24, 128], F32, tag="nat24")
            nc.scalar.dma_start(
                nat24[0:8, :], xrow[0:1, :].rearrange("o (r p) -> o r p", p=128)
            )
            nc.scalar.dma_start(nat24[8:16, :], lc.rearrange("(r p) -> r p", p=128))
            nc.scalar.dma_start(nat24[16:24, :], hprev.rearrange("(r p) -> r p", p=128))
            pkv = pp.tile([128, 48], F32, tag="pa")
            nc.tensor.transpose(pkv[:, 0:24], nat24[:], ident[:24, :24])
            kvecs = ps.tile([128, 24], F32, tag="kvecs")
            nc.vector.tensor_copy(kvecs[:], pkv[:, 0:24])
            # kvecs columns: 0:8 = x k-tiles, 8:16 = lc, 16:24 = h_prev

            # ================= GRU matvecs (row-sharded) =================
            wih_sb = pb.tile([128, 16, GR], F32, tag="wih")
            nc.sync.dma_start(wih_sb[:], wihT.rearrange("(k p) m -> p k m", p=128))
            whh_sb = pb.tile([128, 8, GR], F32, tag="whh")
            nc.sync.dma_start(whh_sb[:], whhT.rearrange("(k p) m -> p k m", p=128))

            pg6 = pp.tile([128, 6], F32, tag="pb")
            for q in range(3):
                for kt in range(16):
                    nc.tensor.matmul(
                        pg6[:, q:q + 1],
                        lhsT=wih_sb[:, kt, q * 128:(q + 1) * 128],
                        rhs=kvecs[:, kt:kt + 1],
                        start=(kt == 0), stop=(kt == 15),
                    )
            for q in range(3):
                for kt in range(8):
                    nc.tensor.matmul(
                        pg6[:, 3 + q:4 + q],
                        lhsT=whh_sb[:, kt, q * 128:(q + 1) * 128],
                        rhs=kvecs[:, 16 + kt:17 + kt],
                        start=(kt == 0), stop=(kt == 7),
                    )
            bias6 = ps.tile([128, 6], F32, tag="bias6")
            nc.scalar.dma_start(bias6[:], gbias.rearrange("(q p) -> p q", p=128))
            gboth = ps.tile([128, 6], F32, tag="gboth")
            nc.vector.tensor_add(gboth[:], pg6[:], bias6[:])
            nc.scalar.dma_start(g_in.rearrange("(q p) -> p q", p=128), gboth[:])

            nc.gpsimd.collective_compute(
                "AllGather", mybir.AluOpType.bypass, replica_groups=GROUPS,
                ins=[g_in[:]], outs=[g_out[:]],
            )

            g48nat = ps.tile([48, 128], F32, tag="g48nat")
            nc.scalar.dma_start(g48nat[:], g_out.rearrange("(r p) -> r p", p=128))
            pg48 = pp.tile([128, 48], F32, tag="pa")
            nc.tensor.transpose(pg48[:], g48nat[:], ident[:48, :48])
            g48sb = ps.tile([128, 48], F32, tag="g48sb")
            nc.vector.tensor_copy(g48sb[:], pg48[:])
            gv = g48sb[:].rearrange("p (c q) -> p c q", q=6)
            i_r, i_z, i_n = gv[:, :, 0], gv[:, :, 1], gv[:, :, 2]
            h_r, h_z, h_n = gv[:, :, 3], gv[:, :, 4], gv[:, :, 5]

            r_sb = ps.tile([128, 8], F32, tag="r_sb")
            z_sb = ps.tile([128, 8], F32, tag="z_sb")
            n_sb = ps.tile([128, 8], F32, tag="n_sb")
            d_sb = ps.tile([128, 8], F32, tag="d_sb")
            hnew = ps.tile([128, 8], F32, tag="hnew")
            h_k = kvecs[:, 16:24]
            nc.vector.tensor_add(r_sb[:], i_r, h_r)
            nc.scalar.activation(r_sb[:], r_sb[:], AF.Sigmoid)
            nc.vector.tensor_add(z_sb[:], i_z, h_z)
            nc.scalar.activation(z_sb[:], z_sb[:], AF.Sigmoid)
            nc.vector.tensor_mul(n_sb[:], r_sb[:], h_n)
            nc.vector.tensor_add(n_sb[:], n_sb[:], i_n)
            nc.scalar.activation(n_sb[:], n_sb[:], AF.Tanh)
            # h_new = n + z*(h - n)
            nc.vector.tensor_sub(d_sb[:], h_k, n_sb[:])
            nc.vector.tensor_mul(d_sb[:], z_sb[:], d_sb[:])
            nc.vector.tensor_add(hnew[:], n_sb[:], d_sb[:])
            nc.scalar.dma_start(out_h.rearrange("(f p) -> p f", p=128), hnew[:])

            # ================= attention (source-sharded) =================
            encT_sb = pb.tile([128, 8, SS], F32, tag="encT")
            nc.sync.dma_start(encT_sb[:], encT.rearrange("(k p) s -> p k s", p=128))
            encn_sb = pb.tile([128, 4, H], F32, tag="encn")
            nc.sync.dma_start(encn_sb[:], encn.rearrange("(s p) k -> p s k", p=128))

            psc = pp.tile([128, 4], F32, tag="pb")
            for m in range(4):
                for kt in range(8):
                    nc.tensor.matmul(
                        psc[:, m:m + 1],
                        lhsT=encT_sb[:, kt, m * 128:(m + 1) * 128],
                        rhs=hnew[:, kt:kt + 1],
                        start=(kt == 0), stop=(kt == 7),
                    )
            # local softmax numerator
            m_c = part_max(psc[:], "sm")
            negm = ps.tile([1, 1], F32, tag="negm")
            nc.scalar.mul(negm[:], m_c[:], -1.0)
            negm_b = bcast128(negm[:], "negm_b")
            e_sb = ps.tile([128, 4], F32, tag="e_sb")
            esum = ps.tile([128, 1], F32, tag="esum")
            nc.scalar.activation(e_sb[:], psc[:], AF.Exp, bias=negm_b[:],
                                 accum_out=esum[:])
            psum_c = psum_sum(esum[:], "sc")
            st2 = ps.tile([1, 2], F32, tag="st2")
            nc.vector.tensor_copy(st2[:, 0:1], m_c[:])
            nc.vector.tensor_copy(st2[:, 1:2], psum_c[:])
            nc.scalar.dma_start(st_in[:], st2[0:1, :])

            nc.gpsimd.collective_compute(
                "AllGather", mybir.AluOpType.bypass, replica_groups=GROUPS,
                ins=[st_in[:]], outs=[st_out[:]],
            )

            st16 = ps.tile([1, 16], F32, tag="st16")
            nc.scalar.dma_start(st16[:], st_out[:])
            stv = st16[0:1, :].rearrange("o (j t) -> o j t", t=2)
            mvec, svec = stv[:, :, 0], stv[:, :, 1]
            Mg = ps.tile([1, 1], F32, tag="Mg")
            nc.vector.reduce_max(Mg[:], mvec, axis=mybir.AxisListType.X)
            negM = ps.tile([1, 1], F32, tag="negM")
            nc.scalar.mul(negM[:], Mg[:], -1.0)
            em8 = ps.tile([1, 8], F32, tag="em8")
            nc.scalar.activation(em8[:], mvec, AF.Exp, bias=negM[:])
            nc.vector.tensor_mul(em8[:], em8[:], svec)
            Sg = ps.tile([1, 1], F32, tag="Sg")
            nc.vector.reduce_sum(Sg[:], em8[:], axis=mybir.AxisListType.X)
            # alpha = exp(m_c - M) / S
            alpha = ps.tile([1, 1], F32, tag="alpha")
            nc.scalar.activation(alpha[:], m_c[:], AF.Exp, bias=negM[:])
            rS = ps.tile([1, 1], F32, tag="rS")
            nc.vector.reciprocal(rS[:], Sg[:])
            nc.vector.tensor_mul(alpha[:], alpha[:], rS[:])
            alpha_b = bcast128(alpha[:], "alpha_b")

            attn_o = ps.tile([128, 4], F32, tag="attn_o")
            nc.vector.tensor_scalar_mul(attn_o[:], e_sb[:], alpha_b[:])
            nc.scalar.dma_start(out_attn.rearrange("(f p) -> p f", p=128), attn_o[:])

            pctx = pp.tile([128, 8], F32, tag="pc")
            for m in range(8):
                for st in range(4):
                    nc.tensor.matmul(
                        pctx[:, m:m + 1],
                        lhsT=encn_sb[:, st, m * 128:(m + 1) * 128],
                        rhs=e_sb[:, st:st + 1],
                        start=(st == 0), stop=(st == 3),
                    )
            ctx_sc = ps.tile([128, 8], F32, tag="ctx_sc")
            nc.vector.tensor_scalar_mul(ctx_sc[:], pctx[:], alpha_b[:])
            nc.scalar.dma_start(ctx_in.rearrange("(f p) -> p f", p=128), ctx_sc[:])

            nc.gpsimd.collective_compute(
                "AllReduce", mybir.AluOpType.add, replica_groups=GROUPS,
                ins=[ctx_in[:]], outs=[ctx_out[:]],
            )
            nc.scalar.dma_start(out_ctx[:], ctx_out[:])
            ctx_k = ps.tile([128, 8], F32, tag="ctx_k")
            nc.scalar.dma_start(ctx_k[:], ctx_out.rearrange("(f p) -> p f", p=128))

            # ================= output projection (vocab-sharded) ============
            l_acc = ps.tile([128, 32], F32, tag="l_acc")
            nc.vector.memset(l_acc[:], 0.0)
            woutv = woutT.rearrange("(k p) m -> p k m", p=128)
            for kt in range(16):
                wt = pw.tile([128, VS], F32, tag="wt")
                nc.sync.dma_start(wt[:], woutv[:, kt, :])
                rhs = hnew[:, kt:kt + 1] if kt < 8 else ctx_k[:, kt - 8:kt - 7]
                pl = pp2.tile([128, 32], F32, tag="pl")
                for m in range(32):
                    msz = 128 if m < VM_FULL else VM_REM
                    nc.tensor.matmul(
                        pl[0:msz, m:m + 1],
                        lhsT=wt[:, m * 128:m * 128 + msz],
                        rhs=rhs,
                        start=True, stop=True,
                    )
                nc.vector.tensor_add(l_acc[:, 0:VM_FULL], l_acc[:, 0:VM_FULL],
                                     pl[:, 0:VM_FULL])
                nc.vector.tensor_add(l_acc[0:VM_REM, VM_FULL:],
                                     l_acc[0:VM_REM, VM_FULL:],
                                     pl[0:VM_REM, VM_FULL:])

            bias_o = ps.tile([128, 32], F32, tag="bias_o")
            nc.vector.memset(bias_o[:], NEG)
            nc.scalar.dma_start(
                bias_o[:, 0:VM_FULL],
                bout[0:VM_FULL * 128].rearrange("(m p) -> p m", p=128),
            )
            nc.scalar.dma_start(
                bias_o[0:VM_REM, VM_FULL:VM_FULL + 1],
                bout[VM_FULL * 128:VS].rearrange("(p o) -> p o", o=1),
            )
            l_sb = ps.tile([128, 32], F32, tag="l_sb")
            nc.vector.tensor_add(l_sb[:], l_acc[:], bias_o[:])

            lm_c = part_max(l_sb[:], "lm")
            neglm = ps.tile([1, 1], F32, tag="neglm")
            nc.scalar.mul(neglm[:], lm_c[:], -1.0)
            neglm_b = bcast128(neglm[:], "neglm_b")
            e2 = ps.tile([128, 32], F32, tag="e2")
            ls128 = ps.tile([128, 1], F32, tag="ls128")
            nc.scalar.activation(e2[:], l_sb[:], AF.Exp, bias=neglm_b[:],
                                 accum_out=ls128[:])
            psum_l = psum_sum(ls128[:], "lsum")
            ls2 = ps.tile([1, 2], F32, tag="ls2")
            nc.vector.tensor_copy(ls2[:, 0:1], lm_c[:])
            nc.vector.tensor_copy(ls2[:, 1:2], psum_l[:])
            nc.scalar.dma_start(ls_in[:], ls2[0:1, :])

            nc.gpsimd.collective_compute(
                "AllGather", mybir.AluOpType.bypass, replica_groups=GROUPS,
                ins=[ls_in[:]], outs=[ls_out[:]],
            )

            ls16 = ps.tile([1, 16], F32, tag="ls16")
            nc.scalar.dma_start(ls16[:], ls_out[:])
            lsv = ls16[0:1, :].rearrange("o (j t) -> o j t", t=2)
            mv2, sv2 = lsv[:, :, 0], lsv[:, :, 1]
            M2 = ps.tile([1, 1], F32, tag="M2")
            nc.vector.reduce_max(M2[:], mv2, axis=mybir.AxisListType.X)
            negM2 = ps.tile([1, 1], F32, tag="negM2")
            nc.scalar.mul(negM2[:], M2[:], -1.0)
            em2 = ps.tile([1, 8], F32, tag="em2")
            nc.scalar.activation(em2[:], mv2, AF.Exp, bias=negM2[:])
            nc.vector.tensor_mul(em2[:], em2[:], sv2)
            S2 = ps.tile([1, 1], F32, tag="S2")
            nc.vector.reduce_sum(S2[:], em2[:], axis=mybir.AxisListType.X)
            lnS = ps.tile([1, 1], F32, tag="lnS")
            nc.scalar.activation(lnS[:], S2[:], AF.Ln)
            logZ = ps.tile([1, 1], F32, tag="logZ")
            nc.vector.tensor_add(logZ[:], M2[:], lnS[:])
            negZ = ps.tile([1, 1], F32, tag="negZ")
            nc.scalar.mul(negZ[:], logZ[:], -1.0)
            negZ_b = bcast128(negZ[:], "negZ_b")

            osb = ps.tile([128, 32], F32, tag="osb")
            nc.scalar.activation(osb[:], l_sb[:], AF.Identity, bias=negZ_b[:])
            poT = pp.tile([32, 128], F32, tag="pc")
            nc.tensor.transpose(poT[:], osb[:], ident[:])
            oT = ps.tile([32, 128], F32, tag="oT")
            nc.vector.tensor_copy(oT[:], poT[:])
            nc.scalar.dma_start(
                out_logits[0:VM_FULL * 128].rearrange("(m p) -> m p", p=128),
                oT[0:VM_FULL, :],
            )
            nc.scalar.dma_start(out_logits[VM_FULL * 128:VS],
                                oT[VM_FULL:VM_FULL + 1, 0:VM_REM])

    nc.compile()
    return nc


_NC_CACHE = None
LAST_RESULT = None


def _get_nc():
    global _NC_CACHE
    if _NC_CACHE is None:
        _NC_CACHE = build_program()
    return _NC_CACHE


def make_in_maps(word, lc, hp, enc, embf, W_ih, W_hh, b_ih, b_hh, W_out, b_out):
    in_maps = []
    for c in range(NCORES):
        rows = np.concatenate(
            [np.arange(g * H + c * 128, g * H + (c + 1) * 128) for g in range(3)]
        )
        in_maps.append({
            "word": word,
            "lc": lc,
            "hprev": hp,
            "emb": embf,
            "wihT": np.ascontiguousarray(W_ih[rows, :].T),
            "whhT": np.ascontiguousarray(W_hh[rows, :].T),
            "gbias": np.concatenate([b_ih[rows], b_hh[rows]]),
            "encT": np.ascontiguousarray(enc[c * SS:(c + 1) * SS, :].T),
            "encn": np.ascontiguousarray(enc[c * SS:(c + 1) * SS, :]),
            "woutT": np.ascontiguousarray(W_out[c * VS:(c + 1) * VS, :].T),
            "bout": np.ascontiguousarray(b_out[c * VS:(c + 1) * VS]),
        })
    return in_maps


def kernel(word_input, last_context, prev_hidden, encoder_outputs,
           emb, W_ih, W_hh, b_ih, b_hh, W_out, b_out, **kw):
    nc = _get_nc()

    word = np.asarray(word_input).reshape(1, 1).astype(np.int32)
    lc = np.ascontiguousarray(np.asarray(last_context, np.float32).reshape(H))
    hp = np.ascontiguousarray(np.asarray(prev_hidden, np.float32).reshape(H))
    enc = np.ascontiguousarray(np.asarray(encoder_outputs, np.float32)[:, 0, :])
    embf = np.ascontiguousarray(np.asarray(emb, np.float32))
    W_ih = np.asarray(W_ih, np.float32)
    W_hh = np.asarray(W_hh, np.float32)
    b_ih = np.asarray(b_ih, np.float32)
    b_hh = np.asarray(b_hh, np.float32)
    W_out = np.asarray(W_out, np.float32)
    b_out = np.asarray(b_out, np.float32)

    in_maps = make_in_maps(word, lc, hp, enc, embf, W_ih, W_hh,
                           b_ih, b_hh, W_out, b_out)

    res = run_bass_kernel_spmd(nc, in_maps, list(range(NCORES)))
    global LAST_RESULT
    LAST_RESULT = res
    results = res.results

    output = np.concatenate([results[c]["out_logits"] for c in range(NCORES)])
    attn_w = np.concatenate([results[c]["out_attn"] for c in range(NCORES)])
    return (
        output.reshape(1, VOCAB).astype(np.float32),
        results[0]["out_ctx"].reshape(1, 1, H).astype(np.float32),
        results[0]["out_h"].reshape(1, 1, H).astype(np.float32),
        attn_w.reshape(1, 1, SRC).astype(np.float32),
    )
